# revision 4
# baseline (speedup 1.0000x reference)
"""Trainium2 Bass kernel for NeighborsValuesAssigner (retrieval_knn).

out[b,:,h,w] = mean_{n in top8} values[n]  where top8 = 8 smallest
dist[b,n,h,w] = 0.5||p_n||^2 - <p_n, x_patch(b,h,w)>  (5x5 'same' conv).

Strategy (8 cores, data-parallel over batch, 4 images/core):
  score[px,n] = <p_n, x_px> - 0.5||p_n||^2   (PE matmul, K=76 w/ bias row)
  per pixel: top-8 values + 9th value via quarter-wise DVE max8 +
  match_replace; threshold t_mid = (t8+t9)/2 (robust to PE rounding).
  pass2 (transposed layout) recomputes score - t_mid via K=77 matmul with
  a -1 coefficient row; ACT Sign gives mask in {-1,+1}; PE mask-matmul:
  psum = sum_n mask*values = 2*sum_top8 - colsum(values).
  out = psum/16 + colsum/16  (ACT Identity w/ per-partition bias).
"""
import os
import sys

sys.path.insert(0, "/opt/trn_rl_repo")

import numpy as np

B, C, H, W = 32, 3, 64, 64
N, D = 2048, 128
KH = KW = 5
KDIM = C * KH * KW          # 75
NCORES = 8
BLOC = B // NCORES          # 4 images per core
PX = BLOC * H * W           # 16384 pixels per core
GPX = 512                   # pixels per group
NGRP = PX // GPX            # 32 groups per core
NCHUNK = N // 128           # 16 patch chunks

_CACHE = {}


def _build_program(ngroups=NGRP):
    import concourse.bacc as bacc
    import concourse.tile as tile
    import concourse.mybir as mybir
    from contextlib import ExitStack

    f32 = mybir.dt.float32
    nc = bacc.Bacc("TRN2", target_bir_lowering=False, debug=False)

    xp = nc.dram_tensor("xp", [77, PX], f32, kind="ExternalInput").ap()
    p77 = nc.dram_tensor("p77", [77, N], f32, kind="ExternalInput").ap()
    vsb = nc.dram_tensor("vsb", [128, N], f32, kind="ExternalInput").ap()
    b16 = nc.dram_tensor("b16", [128, 1], f32, kind="ExternalInput").ap()
    ihalf = nc.dram_tensor("ihalf", [128, 128], f32, kind="ExternalInput").ap()
    out = nc.dram_tensor("out", [BLOC, 128, H * W], f32, kind="ExternalOutput").ap()

    with tile.TileContext(nc) as tc, ExitStack() as ctx:
        const = ctx.enter_context(tc.tile_pool(name="const", bufs=1))
        xpp = ctx.enter_context(tc.tile_pool(name="xpp", bufs=3))
        mhp = ctx.enter_context(tc.tile_pool(name="mhp", bufs=3))
        smp = ctx.enter_context(tc.tile_pool(name="smp", bufs=3))
        mkp = ctx.enter_context(tc.tile_pool(name="mkp", bufs=3))
        otp = ctx.enter_context(tc.tile_pool(name="otp", bufs=2))
        ps1 = ctx.enter_context(tc.tile_pool(name="ps1", bufs=4, space="PSUM"))
        psA = ctx.enter_context(tc.tile_pool(name="psA", bufs=2, space="PSUM"))
        psB = ctx.enter_context(tc.tile_pool(name="psB", bufs=1, space="PSUM"))
        ptp = ctx.enter_context(tc.tile_pool(name="ptp", bufs=1, space="PSUM"))

        p77_t = const.tile([77, N], f32)
        vsb_t = const.tile([128, N], f32)
        b16_t = const.tile([128, 1], f32)
        ih_t = const.tile([128, 128], f32)
        nc.sync.dma_start(p77_t[:], p77[:])
        nc.sync.dma_start(vsb_t[:], vsb[:])
        nc.sync.dma_start(b16_t[:], b16[:])
        nc.sync.dma_start(ih_t[:], ihalf[:])

        grp_per_img = (H * W) // GPX  # 8
        for g in range(ngroups):
            b, s = divmod(g, grp_per_img)  # image b, 512-px slice s
            xp_t = xpp.tile([77, GPX], f32, tag="xp")
            nc.sync.dma_start(xp_t[0:76, :], xp[0:76, g * GPX:(g + 1) * GPX])

            ptr = ptp.tile([1, GPX], f32, tag="ptr")
            for t in range(4):  # 128-pixel tiles within the group
                lhs = xp_t[0:76, t * 128:(t + 1) * 128]
                mh = mhp.tile([128, 32], f32, tag="mh")
                for q in range(4):  # quarter chunks of the 2048 patches
                    p1 = ps1.tile([128, 512], f32, tag="p1")
                    nc.tensor.matmul(
                        p1[:], lhs, p77_t[0:76, q * 512:(q + 1) * 512],
                        start=True, stop=True)
                    nc.vector.max(mh[:, q * 8:(q + 1) * 8], p1[:])
                m8 = smp.tile([128, 8], f32, tag="m8")
                nc.vector.max(m8[:], mh[:])
                mh2 = mhp.tile([128, 32], f32, tag="mh2")
                nc.vector.match_replace(mh2[:], m8[:], mh[:], -1e30)
                m9 = smp.tile([128, 8], f32, tag="m9")
                nc.vector.max(m9[:], mh2[:])
                tsum = smp.tile([128, 1], f32, tag="tsum")
                nc.vector.tensor_add(tsum[:], m8[:, 7:8], m9[:, 0:1])
                # transpose t_mid = 0.5*(t8+t9) into a row of ptr
                nc.tensor.matmul(
                    ptr[0:1, t * 128:(t + 1) * 128], tsum[:], ih_t[:],
                    start=True, stop=True)

            # t_mid row: PSUM -> SBUF (ACT), then DMA to partition 76 of xp
            trow = otp.tile([1, GPX], f32, tag="trow")
            nc.scalar.copy(trow[0:1, :], ptr[0:1, :])
            nc.sync.dma_start(xp_t[76:77, :], trow[0:1, :])

            pB = psB.tile([128, GPX], f32, tag="pB")
            for c in range(NCHUNK):
                pA = psA.tile([128, GPX], f32, tag="pA")
                nc.tensor.matmul(
                    pA[:], p77_t[0:77, c * 128:(c + 1) * 128], xp_t[:, :],
                    start=True, stop=True)
                mk = mkp.tile([128, GPX], f32, tag="mk")
                nc.scalar.sign(mk[:], pA[:])
                nc.tensor.matmul(
                    pB[:], vsb_t[:, c * 128:(c + 1) * 128], mk[:],
                    start=(c == 0), stop=(c == NCHUNK - 1))

            ot = otp.tile([128, GPX], f32, tag="ot")
            nc.scalar.activation(
                ot[:], pB[:], mybir.ActivationFunctionType.Identity,
                bias=b16_t[:], scale=1.0 / 16.0)
            nc.sync.dma_start(out[b, :, s * GPX:(s + 1) * GPX], ot[:])

    nc.compile()
    return nc


def _get_program():
    key = "nc"
    if key not in _CACHE:
        _CACHE[key] = _build_program()
    return _CACHE[key]


def _im2col(x):
    """x: (B,3,64,64) f32 -> cols (B, 75, 4096) f32, k=(c,dy,dx), px=(h,w)."""
    xpad = np.pad(x, ((0, 0), (0, 0), (2, 2), (2, 2)))
    win = np.lib.stride_tricks.sliding_window_view(xpad, (KH, KW), axis=(2, 3))
    # win: (B, 3, 64, 64, 5, 5) -> (B, 3, 5, 5, 64, 64)
    cols = np.ascontiguousarray(win.transpose(0, 1, 4, 5, 2, 3))
    return cols.reshape(x.shape[0], KDIM, H * W)


def kernel(x, patches, values):
    from concourse.bass_utils import run_bass_kernel_spmd

    x = np.asarray(x, dtype=np.float32)
    patches = np.asarray(patches, dtype=np.float32)
    values = np.asarray(values, dtype=np.float32)

    nc = _get_program()

    pf = patches.reshape(N, KDIM)
    p77 = np.zeros((77, N), np.float32)
    p77[0:KDIM] = pf.T
    p77[75] = (-0.5 * np.sum(pf.astype(np.float64) ** 2, axis=1)).astype(np.float32)
    p77[76] = -1.0

    vsb = np.ascontiguousarray(
        values.reshape(NCHUNK, 128, 128).transpose(1, 0, 2).reshape(128, N))
    colsum = np.sum(values.astype(np.float64), axis=0)
    b16 = (colsum / 16.0).astype(np.float32).reshape(128, 1)
    ihalf = (np.eye(128, dtype=np.float32) * 0.5)

    cols = _im2col(x)  # (32, 75, 4096)
    in_maps = []
    for i in range(NCORES):
        xp = np.zeros((77, PX), np.float32)
        xp[0:KDIM] = np.concatenate(
            [cols[i * BLOC + j] for j in range(BLOC)], axis=1)
        xp[75] = 1.0
        in_maps.append({"xp": xp, "p77": p77, "vsb": vsb, "b16": b16,
                        "ihalf": ihalf})

    res = run_bass_kernel_spmd(nc, in_maps, list(range(NCORES)))

    out = np.empty((B, D, H, W), np.float32)
    for i in range(NCORES):
        o = res.results[i]["out"]  # (BLOC, 128, 4096)
        out[i * BLOC:(i + 1) * BLOC] = o.reshape(BLOC, D, H, W)
    return out


# revision 10
# speedup vs baseline: 498.2148x; 498.2148x over previous
"""Trainium2 Bass kernel for NeighborsValuesAssigner (retrieval_knn).

out[b,:,h,w] = mean_{n in top8} values[n]  where top8 = 8 smallest
dist[b,n,h,w] = 0.5||p_n||^2 - <p_n, x_patch(b,h,w)>  (5x5 'same' conv).

8 cores, data-parallel over batch (4 images/core). Per core:
  pass1  score[px,n] = <p_n,x_px> - 0.5||p_n||^2 on PE as 3 accumulating
         fp16 matmuls (hi/lo fp16 split: xh@ph + xh@pl + xl@ph; error ~2^-22, below fp32 rounding).
  top8   DVE max8 over each [128,1024] PSUM half + merge -> t8 = 8th value.
  mask   DVE tensor_scalar is_ge(score, t8) straight from PSUM -> fp16
         {0,1} mask [px, n] (exact: same-arithmetic inclusive compare).
  maskT  PE transpose (fp16, via identity) -> PSUM -> ACT drain to SBUF.
  matmul out[D,px] = sum_n values[n,D]*maskT[n,px] (fp16 operands,
         fp32 PSUM accumulation over 16 chunks of n).
  final  ACT scale 1/8 -> DMA to DRAM (output is D-major: perfect layout).
"""
import sys

sys.path.insert(0, "/opt/trn_rl_repo")

import numpy as np
import ml_dtypes

B, C, H, W = 32, 3, 64, 64
N, D = 2048, 128
KH = KW = 5
KDIM = C * KH * KW          # 75
KROWS = KDIM + 1            # 76 = patch dims + bias/ones row
NCORES = 8
BLOC = B // NCORES          # 4 images per core
PX = BLOC * H * W           # 16384 pixels per core
GPX = 512                   # pixels per group
NGRP = PX // GPX            # 32 groups per core
NCHUNK = N // 128           # 16 patch chunks

BF16 = ml_dtypes.bfloat16
_CACHE = {}


def _build_program(loop_r=0):
    """loop_r=0: straight-line. loop_r>0: wrap body in a device-side
    For_i loop running it loop_r times (for HW timing via wall deltas)."""
    import concourse.bacc as bacc
    import concourse.tile as tile
    import concourse.mybir as mybir
    from contextlib import ExitStack

    f32 = mybir.dt.float32
    f16 = mybir.dt.float16
    bf16 = mybir.dt.bfloat16
    nc = bacc.Bacc("TRN2", target_bir_lowering=False, debug=False)

    xph = nc.dram_tensor("xph", [KROWS, PX], f16, kind="ExternalInput").ap()
    xpl = nc.dram_tensor("xpl", [KROWS, PX], f16, kind="ExternalInput").ap()
    ph = nc.dram_tensor("ph", [KROWS, N], f16, kind="ExternalInput").ap()
    pl = nc.dram_tensor("pl", [KROWS, N], f16, kind="ExternalInput").ap()
    vs16 = nc.dram_tensor("vs16", [128, N], f16, kind="ExternalInput").ap()
    id16 = nc.dram_tensor("id16", [128, 128], f16, kind="ExternalInput").ap()
    out = nc.dram_tensor("out", [BLOC, 128, H * W], f32, kind="ExternalOutput").ap()

    with tile.TileContext(nc) as tc, ExitStack() as ctx:
        const = ctx.enter_context(tc.tile_pool(name="const", bufs=1))
        xpp = ctx.enter_context(tc.tile_pool(name="xpp", bufs=3))
        mhp = ctx.enter_context(tc.tile_pool(name="mhp", bufs=3))
        mkp = ctx.enter_context(tc.tile_pool(name="mkp", bufs=6))
        mtp = ctx.enter_context(tc.tile_pool(name="mtp", bufs=3))
        otp = ctx.enter_context(tc.tile_pool(name="otp", bufs=2))
        ps1 = ctx.enter_context(tc.tile_pool(name="ps1", bufs=2, space="PSUM"))
        pst = ctx.enter_context(tc.tile_pool(name="pst", bufs=2, space="PSUM"))
        psB = ctx.enter_context(tc.tile_pool(name="psB", bufs=2, space="PSUM"))

        ph_t = const.tile([KROWS, N], f16)
        pl_t = const.tile([KROWS, N], f16)
        vs_t = const.tile([128, N], f16)
        id_t = const.tile([128, 128], f16)
        nc.sync.dma_start(ph_t[:], ph[:])
        nc.sync.dma_start(pl_t[:], pl[:])
        nc.sync.dma_start(vs_t[:], vs16[:])
        nc.sync.dma_start(id_t[:], id16[:])

        loop_cm = tc.For_i(0, loop_r, 1) if loop_r else None
        if loop_cm is not None:
            loop_cm.__enter__()

        grp_per_img = (H * W) // GPX  # 8
        for g in range(NGRP):
            b, s = divmod(g, grp_per_img)
            xh_t = xpp.tile([KROWS, GPX], f16, tag="xh")
            xl_t = xpp.tile([KROWS, GPX], f16, tag="xl")
            nc.sync.dma_start(xh_t[:], xph[:, g * GPX:(g + 1) * GPX])
            nc.sync.dma_start(xl_t[:], xpl[:, g * GPX:(g + 1) * GPX])

            masks = []
            for t in range(4):  # 128-px tiles in the group
                lh = xh_t[:, t * 128:(t + 1) * 128]
                ll = xl_t[:, t * 128:(t + 1) * 128]
                mh = mhp.tile([128, 16], f32, tag="mh")
                m8 = mhp.tile([128, 8], f32, tag="m8")
                mk = mkp.tile([128, N], f16, tag="mk")
                halves = []
                for h in range(2):
                    p1 = ps1.tile([128, 1024], f32, tag="p1")
                    for q in range(2):  # N=512 per matmul (one PSUM bank)
                        rsl = slice(h * 1024 + q * 512, h * 1024 + (q + 1) * 512)
                        osl = slice(q * 512, (q + 1) * 512)
                        nc.tensor.matmul(p1[:, osl], lh, ph_t[:, rsl],
                                         start=True, stop=False)
                        nc.tensor.matmul(p1[:, osl], lh, pl_t[:, rsl],
                                         start=False, stop=False)
                        nc.tensor.matmul(p1[:, osl], ll, ph_t[:, rsl],
                                         start=False, stop=True)
                    nc.vector.max(mh[:, h * 8:(h + 1) * 8], p1[:])
                    halves.append(p1)
                nc.vector.max(m8[:], mh[:])
                for h in range(2):
                    nc.vector.tensor_scalar(
                        mk[:, h * 1024:(h + 1) * 1024], halves[h][:],
                        m8[:, 7:8], None, mybir.AluOpType.is_ge)
                masks.append(mk)

            pB = psB.tile([128, GPX], f32, tag="pB")
            for c in range(NCHUNK):
                pt = pst.tile([128, GPX], f16, tag="pt")
                for t in range(4):
                    nc.tensor.transpose(
                        pt[:, t * 128:(t + 1) * 128],
                        masks[t][:, c * 128:(c + 1) * 128], id_t[:])
                mt = mtp.tile([128, GPX], f16, tag="mt")
                nc.scalar.copy(mt[:], pt[:])
                nc.tensor.matmul(
                    pB[:], vs_t[:, c * 128:(c + 1) * 128], mt[:],
                    start=(c == 0), stop=(c == NCHUNK - 1))

            ot = otp.tile([128, GPX], f32, tag="ot")
            nc.scalar.mul(ot[:], pB[:], 0.125)
            nc.sync.dma_start(out[b, :, s * GPX:(s + 1) * GPX], ot[:])

        if loop_cm is not None:
            loop_cm.__exit__(None, None, None)

    nc.compile()
    return nc


def _get_program():
    if "nc" not in _CACHE:
        _CACHE["nc"] = _build_program()
    return _CACHE["nc"]


def _im2col(x):
    """x: (B,3,64,64) f32 -> cols (B, 75, 4096) f32, k=(c,dy,dx), px=(h,w)."""
    xpad = np.pad(x, ((0, 0), (0, 0), (2, 2), (2, 2)))
    win = np.lib.stride_tricks.sliding_window_view(xpad, (KH, KW), axis=(2, 3))
    cols = np.ascontiguousarray(win.transpose(0, 1, 4, 5, 2, 3))
    return cols.reshape(x.shape[0], KDIM, H * W)


def _host_prep(x, patches, values):
    """Returns per-core in_maps list."""
    pf = patches.reshape(N, KDIM)
    bias = (-0.5 * np.sum(pf.astype(np.float64) ** 2, axis=1)).astype(np.float32)

    pfull = np.zeros((KROWS, N), np.float32)
    pfull[0:KDIM] = pf.T
    pfull[KDIM] = bias
    ph = pfull.astype(np.float16)
    pl = (pfull - ph.astype(np.float32)).astype(np.float16)

    vs16 = np.ascontiguousarray(
        values.reshape(NCHUNK, 128, 128).transpose(1, 0, 2).reshape(128, N)
    ).astype(np.float16)
    id16 = np.eye(128, dtype=np.float16)

    cols = _im2col(x)  # (32, 75, 4096) f32
    in_maps = []
    for i in range(NCORES):
        xfull = np.empty((KROWS, PX), np.float32)
        xfull[0:KDIM] = np.concatenate(
            [cols[i * BLOC + j] for j in range(BLOC)], axis=1)
        xfull[KDIM] = 1.0
        xh = xfull.astype(np.float16)
        xl = (xfull - xh.astype(np.float32)).astype(np.float16)
        in_maps.append({"xph": xh, "xpl": xl, "ph": ph, "pl": pl,
                        "vs16": vs16, "id16": id16})
    return in_maps


def kernel(x, patches, values):
    from concourse.bass_utils import run_bass_kernel_spmd

    x = np.asarray(x, dtype=np.float32)
    patches = np.asarray(patches, dtype=np.float32)
    values = np.asarray(values, dtype=np.float32)

    nc = _get_program()
    in_maps = _host_prep(x, patches, values)
    res = run_bass_kernel_spmd(nc, in_maps, list(range(NCORES)))

    out = np.empty((B, D, H, W), np.float32)
    for i in range(NCORES):
        o = res.results[i]["out"]  # (BLOC, 128, 4096)
        out[i * BLOC:(i + 1) * BLOC] = o.reshape(BLOC, D, H, W)
    return out
